# revision 12
# baseline (speedup 1.0000x reference)
"""Causal self-attention (B=2, T=2048, EMB=1024, 16 heads) on 8 TRN2 NeuronCores.

Sharding: core c handles batch c//4 and heads [4*(c%4), 4*(c%4)+4).
 - Wqkv is split column-wise per head group (q part pre-scaled by 1/sqrt(hd)),
 - Wproj is split row-wise per head group,
 - each core emits a partial [2048, 1024] projection output,
 - host sums the 4 partials per batch and adds bproj + bv@Wproj
   (softmax rows sum to 1, so the v bias contributes a constant row vector
   that the host can add; the device kernel drops bv entirely).

Device kernel (per core, SPMD):
 - host supplies x^T so both qkv matmul operands have the contraction on
   partitions; qkT is produced directly in [qkv_col, token] (transposed)
   layout. v is produced token-major directly (lhsT = x^T token tile), no
   PE transposes needed, and lands next to a ones column (row-sum trick
   for the softmax denominator).
 - attention runs in the S^T = (K Q^T) layout, chunk-major. Within a chunk
   the two head-pair units are interleaved j-tile by j-tile so four
   independent S->exp chains are always in flight; PV matmuls trail one
   block of 4 j-tiles as contiguous per-head accumulation chains (avoids
   the HW per-matmul PSUM-group-switch cost).
 - at chunk end each [65, 512] PV accumulator (row 64 = softmax denominator
   via the ones column) is evacuated to SBUF, freeing all four PSUM banks
   for the next chunk while the normalize/projection epilogue is deferred.
 - deferred work (next chunk's stage-1 pieces first, then epilogue
   normalize + projection pieces) drains one piece per j-step through the
   attention loops, keeping PE busy during exp waits.
 - softmax denominator DMAs (partition spread for the 128-lane reciprocal)
   ride the gpsimd/Pool SWDGE queue so they never block the activation queue.

All matmul operands are f16 (~1e-4 rel err); PSUM accumulation in f32.
"""
import sys

sys.path.insert(0, "/opt/trn_rl_repo")

import numpy as np

B = 2
T = 2048
EMB = 1024
HEADS = 16
HD = EMB // HEADS  # 64
NCORES = 8
GROUPS = 4                 # head groups (cores per batch)
HPC = HEADS // GROUPS      # 4 heads per core
CQ = HPC * HD              # 256 q (or k or v) columns per core
KT = EMB // 128            # 8 contraction tiles
TCH = 512                  # token chunk
NCH = T // TCH             # 4 chunks
NTT = T // 128             # 16 token tiles
NR = CQ // 128             # 2 head-dim row tiles (= head pairs)
SCALE = HD ** -0.5

_compiled = {}
ABLATE = None  # None | 's1' (stage1 only) | 's12' (no projection)


def _build(loop=1):
    import concourse.bass as bass
    import concourse.tile as tile
    from concourse import bacc, mybir

    F32 = mybir.dt.float32
    F16 = mybir.dt.float16
    AF = mybir.ActivationFunctionType

    nc = bacc.Bacc(None, target_bir_lowering=False)
    xT = nc.dram_tensor("xT", [EMB, T], F16, kind="ExternalInput")
    wqkv = nc.dram_tensor("wqkv", [EMB, 3 * CQ], F16, kind="ExternalInput")
    bqkv = nc.dram_tensor("bqkv", [128, 4], F32, kind="ExternalInput")
    wproj = nc.dram_tensor("wproj", [CQ, EMB], F16, kind="ExternalInput")
    out = nc.dram_tensor("out", [T, EMB], F32, kind="ExternalOutput")

    xT_r = xT.rearrange("(kt p) t -> p kt t", p=128)
    wqkv_r = wqkv.rearrange("(kt p) c -> p kt c", p=128)
    wproj_r = wproj.rearrange("(r p) e -> p r e", p=128)

    with tile.TileContext(nc) as tc:
        with (
            tc.tile_pool(name="const", bufs=1) as const,
            tc.tile_pool(name="qk", bufs=1) as qkp,
            tc.tile_pool(name="xt", bufs=3) as xtp,
            tc.tile_pool(name="pt", bufs=20) as ptp,
            tc.tile_pool(name="oh", bufs=1) as ohp,
            tc.tile_pool(name="csb", bufs=8) as csbp,
            tc.tile_pool(name="den", bufs=5) as denp,
            tc.tile_pool(name="osb", bufs=3) as osbp,
            tc.tile_pool(name="ps", bufs=4, space="PSUM") as psS,
            tc.tile_pool(name="psO", bufs=4, space="PSUM") as psO,
        ):
            # ---- constants ----
            # weights on the scalar HWDGE queue, per k-tile, so the sync
            # queue's xt chunk loads run in parallel and matmuls start early
            bias_sb = const.tile([128, 4], F32)
            nc.scalar.dma_start(out=bias_sb, in_=bqkv[:, :])
            w_sb = const.tile([128, KT, 3 * CQ], F16)
            for kt in range(KT):
                nc.scalar.dma_start(
                    out=w_sb[:, kt, 0:CQ], in_=wqkv_r[:, kt, 0:CQ]
                )
            for cp in range(1, 3):
                nc.scalar.dma_start(
                    out=w_sb[:, :, cp * CQ : (cp + 1) * CQ],
                    in_=wqkv_r[:, :, cp * CQ : (cp + 1) * CQ],
                )
            # stage-3 weights loaded inside body() after the xt chunks
            wp_sb = const.tile([128, NR, EMB], F16)
            tri_f = const.tile([128, 128], F32)
            nc.gpsimd.memset(tri_f, 1.0)
            # keep where i(free) >= j(partition): -j + i >= 0
            nc.gpsimd.affine_select(
                out=tri_f, in_=tri_f,
                compare_op=mybir.AluOpType.is_ge,
                fill=0.0, base=0,
                pattern=[[1, 128]], channel_multiplier=-1,
            )
            tri = const.tile([128, 128], F16)
            nc.vector.tensor_copy(tri, tri_f)
            ones_f = const.tile([128, 64], F32)
            nc.vector.memset(ones_f, 1.0)
            ones64 = const.tile([1, 64], F16)
            nc.vector.tensor_copy(ones64, ones_f[0:1, :])
            # v in token-major, per (token_tile, head): 64 cols + ones col
            v_sb = const.tile([128, NTT, HPC, HD + 1], F16)
            nc.vector.tensor_copy(
                out=v_sb[:, :, :, HD : HD + 1],
                in_=ones_f.rearrange("p (a b c) -> p a b c", a=NTT, b=HPC),
            )
            qkT_sb = qkp.tile([128, 4, T], F16)
            ohT = ohp.tile([128, NR, T], F16)

            def body():
                # two-priority deferred-work queues: (chunk, fn) stage-1
                # pieces first, then epilogue pieces
                q_s1 = []
                q_epi = []

                def emit_filler(n=1):
                    for _ in range(n):
                        if q_s1:
                            q_s1.pop(0)[1]()
                        elif q_epi:
                            q_epi.pop(0)()
                        else:
                            return

                def force_s1(upto_ch):
                    while q_s1 and q_s1[0][0] <= upto_ch:
                        q_s1.pop(0)[1]()

                # ---- stage 1: qkv projection ----
                # qkT_sb[:, cb, t]: cb 0,1 = q col-tiles, 2,3 = k col-tiles
                # (transposed layout); v goes token-major straight into v_sb
                def stage1_chunk(ch, inline):
                    xt = xtp.tile([128, KT, TCH], F16)
                    if ch == 0:
                        # per k-tile loads so the first matmul starts after
                        # the first [128, 512] tile instead of the full chunk
                        for kt in range(KT):
                            nc.sync.dma_start(
                                out=xt[:, kt, :],
                                in_=xT_r[:, kt, ch * TCH : (ch + 1) * TCH],
                            )
                    else:
                        nc.sync.dma_start(
                            out=xt, in_=xT_r[:, :, ch * TCH : (ch + 1) * TCH]
                        )

                    def make_qk(cb):
                        def qk_piece():
                            ps = psS.tile([128, TCH], mybir.dt.float32, tag="ps")
                            for kt in range(KT):
                                nc.tensor.matmul(
                                    ps,
                                    w_sb[:, kt, cb * 128 : (cb + 1) * 128],
                                    xt[:, kt, :],
                                    start=(kt == 0),
                                    stop=(kt == KT - 1),
                                )
                            nc.vector.tensor_scalar_add(
                                qkT_sb[:, cb, ch * TCH : (ch + 1) * TCH],
                                ps,
                                bias_sb[:, cb : cb + 1],
                            )
                        return qk_piece

                    def make_v(s):
                        def v_piece():
                            tt = ch * (TCH // 128) + s
                            psv = psS.tile([128, CQ], mybir.dt.float32, tag="ps")
                            for kt in range(KT):
                                nc.tensor.matmul(
                                    psv,
                                    xt[:, kt, s * 128 : (s + 1) * 128],
                                    w_sb[:, kt, 2 * CQ : 3 * CQ],
                                    start=(kt == 0),
                                    stop=(kt == KT - 1),
                                )
                            nc.vector.tensor_copy(
                                v_sb[:, tt, :, 0:HD],
                                psv.rearrange("p (h d) -> p h d", h=HPC),
                            )
                        return v_piece

                    pieces = [make_qk(cb) for cb in range(4)]
                    pieces += [make_v(s) for s in range(TCH // 128)]
                    if inline:
                        for p in pieces:
                            p()
                    else:
                        q_s1.extend((ch, p) for p in pieces)

                def make_partB_rp(rec_rows, holder):
                    def partB_rp():
                        # packed reciprocal broadcast: head-even -> psum rows
                        # 0:64, head-odd -> rows 64:128
                        rp = psS.tile([128, TCH], mybir.dt.float32, tag="ps")
                        nc.tensor.matmul(
                            rp[0:64, :], ones64, rec_rows[0], start=True, stop=True
                        )
                        nc.tensor.matmul(
                            rp[64:128, :], ones64, rec_rows[1], start=True, stop=True
                        )
                        rec_sb = denp.tile([64, 2, TCH], F32, tag="rec_sb")
                        nc.vector.tensor_copy(rec_sb[:, 0, :], rp[0:64, :])
                        nc.vector.tensor_copy(rec_sb[:, 1, :], rp[64:128, :])
                        holder.append(rec_sb)
                    return partB_rp

                def make_partB_mul(cs2, holder, r, base):
                    def partB_mul():
                        rec_sb = holder[0]
                        nc.vector.tensor_mul(
                            ohT[0:64, r, base : base + TCH],
                            cs2[0][0:64, :],
                            rec_sb[:, 0, :],
                        )
                        nc.vector.tensor_mul(
                            ohT[64:128, r, base : base + TCH],
                            cs2[1][0:64, :],
                            rec_sb[:, 1, :],
                        )
                    return partB_mul

                def make_proj(tt, nn):
                    def proj_piece():
                        pp = psS.tile([128, TCH], mybir.dt.float32, tag="ps")
                        for r2 in range(NR):
                            nc.tensor.matmul(
                                pp,
                                ohT[:, r2, tt * 128 : (tt + 1) * 128],
                                wp_sb[:, r2, nn * 512 : (nn + 1) * 512],
                                start=(r2 == 0),
                                stop=(r2 == NR - 1),
                            )
                        osb = osbp.tile([128, 512], F32)
                        nc.any.tensor_copy(osb, pp)
                        nc.sync.dma_start(
                            out=out[
                                tt * 128 : (tt + 1) * 128,
                                nn * 512 : (nn + 1) * 512,
                            ],
                            in_=osb,
                        )
                    return proj_piece

                def emit_chunk(cc):
                    base = cc * TCH
                    jmax = 4 * cc + 3
                    diag = [j for j in range(4 * cc, jmax + 1) if j != 0]
                    rest = [j for j in range(1, 4 * cc)]
                    order = [0] + diag + rest
                    jlast = order[-1]
                    psC = [
                        [
                            psO.tile(
                                [65, TCH], mybir.dt.float32, tag="psO",
                                name=f"psC_{cc}_{_r}_{_hh}",
                            )
                            for _hh in range(2)
                        ]
                        for _r in range(NR)
                    ]
                    blks = [[] for _ in range(NR)]

                    def pv_chain(r, blk):
                        for hh in range(2):
                            for jt, pts, lo, hi in blk:
                                nc.tensor.matmul(
                                    psC[r][hh][:, lo - base : hi - base],
                                    v_sb[:, jt, 2 * r + hh, :],
                                    pts[hh][:, 0 : hi - lo],
                                    start=(jt == 0),
                                    stop=(jt == jlast),
                                    skip_group_check=(jt != 0),
                                )

                    for pos, jt in enumerate(order):
                        i0 = 128 * jt
                        lo = max(base, i0)
                        hi = base + TCH
                        w = hi - lo
                        for r in range(NR):
                            pts = []
                            for hh in range(2):
                                po = 64 * hh
                                sp = psS.tile(
                                    [128, TCH], mybir.dt.float32, tag="ps"
                                )
                                nc.tensor.matmul(
                                    sp[:, 0:w],
                                    qkT_sb[po : po + 64, 2 + r, i0 : i0 + 128],
                                    qkT_sb[po : po + 64, r, lo:hi],
                                    start=True,
                                    stop=True,
                                )
                                pt = ptp.tile([128, TCH], F16)
                                nc.scalar.activation(
                                    pt[:, 0:w], sp[:, 0:w], AF.Exp
                                )
                                if i0 >= base:  # diagonal block: causal mask
                                    nc.vector.tensor_mul(
                                        pt[:, 0:128], pt[:, 0:128], tri
                                    )
                                pts.append(pt)
                            blks[r].append((jt, pts, lo, hi))
                            if len(blks[r]) == 4 and pos < len(order) - 1:
                                pv_chain(r, blks[r])
                                blks[r] = []
                        emit_filler(1)
                    for r in range(NR):
                        pv_chain(r, blks[r])
                    # evacuate accumulators to SBUF (frees PSUM), then the
                    # denominator chains (no PE work; DMAs on the Pool queue,
                    # reading the den row straight from PSUM so they start in
                    # parallel with the evacuation copy)
                    all_cs2 = []
                    all_rec = []
                    for r in range(NR):
                        cs2 = []
                        rec_rows = []
                        for hh in range(2):
                            csb = csbp.tile([65, TCH], F32)
                            nc.vector.tensor_copy(csb, psC[r][hh])
                            cs2.append(csb)
                            den128 = denp.tile([128, TCH // 128], F32, tag="den128")
                            nc.gpsimd.dma_start(out=den128, in_=csb[64:65, :])
                            rec128 = denp.tile([128, TCH // 128], F32, tag="rec128")
                            nc.vector.reciprocal(rec128, den128)
                            rec16 = denp.tile([128, TCH // 128], F16, tag="rec16")
                            nc.vector.tensor_copy(rec16, rec128)
                            rec_row = denp.tile([1, TCH], F16, tag="rec_row")
                            nc.gpsimd.dma_start(out=rec_row, in_=rec16)
                            rec_rows.append(rec_row)
                        all_cs2.append(cs2)
                        all_rec.append(rec_rows)
                    last = cc == NCH - 1
                    holders = []
                    for r in range(NR):
                        holder = []
                        holders.append(holder)
                        if last:
                            make_partB_rp(all_rec[r], holder)()
                        else:
                            q_epi.append(make_partB_rp(all_rec[r], holder))
                            q_epi.append(
                                make_partB_mul(all_cs2[r], holder, r, base)
                            )
                    if last:
                        # inline tail: per token-tile, normalize-muls then the
                        # projection chain, so DVE and PE pipeline
                        for tt in range(4 * cc, 4 * cc + 4):
                            o0 = tt * 128
                            for r in range(NR):
                                rec_sb = holders[r][0]
                                for hh in range(2):
                                    nc.vector.tensor_mul(
                                        ohT[
                                            64 * hh : 64 * hh + 64,
                                            r,
                                            o0 : o0 + 128,
                                        ],
                                        all_cs2[r][hh][
                                            0:64, o0 - base : o0 - base + 128
                                        ],
                                        rec_sb[:, hh, o0 - base : o0 - base + 128],
                                    )
                            if ABLATE != "s12":
                                for nn in range(EMB // 512):
                                    make_proj(tt, nn)()
                    elif ABLATE != "s12":
                        for tt in range(4 * cc, 4 * cc + 4):
                            for nn in range(EMB // 512):
                                q_epi.append(make_proj(tt, nn))

                # ---- emission schedule ----
                stage1_chunk(0, inline=True)
                if ABLATE == "s1":
                    for ch in range(1, NCH):
                        stage1_chunk(ch, inline=True)
                    return
                stage1_chunk(1, inline=False)
                nc.sync.dma_start(out=wp_sb, in_=wproj_r)
                emit_chunk(0)
                stage1_chunk(2, inline=False)
                force_s1(1)
                emit_chunk(1)
                stage1_chunk(3, inline=False)
                force_s1(2)
                emit_chunk(2)
                force_s1(3)
                emit_chunk(3)
                while q_s1 or q_epi:
                    emit_filler(1)

            if loop == 1:
                body()
            else:
                with tc.For_i(
                    0, loop, 1,
                    hint_engines=(
                        mybir.EngineType.PE,
                        mybir.EngineType.Activation,
                        mybir.EngineType.DVE,
                        mybir.EngineType.SP,
                        mybir.EngineType.Pool,
                    ),
                ):
                    body()

    nc.finalize()
    return nc


def _shard_inputs(x, Wqkv, bqkv, Wproj):
    """Build the 8 per-core input maps."""
    x = np.asarray(x, dtype=np.float32)
    Wqkv = np.asarray(Wqkv, dtype=np.float32)
    bqkv = np.asarray(bqkv, dtype=np.float32)
    Wproj = np.asarray(Wproj, dtype=np.float32)

    in_maps = []
    for c in range(NCORES):
        b = c // GROUPS
        g = c % GROUPS
        cols = slice(g * CQ, (g + 1) * CQ)
        wq = Wqkv[:, cols] * SCALE
        wk = Wqkv[:, EMB:][:, cols]
        wv = Wqkv[:, 2 * EMB:][:, cols]
        w_c = np.ascontiguousarray(
            np.concatenate([wq, wk, wv], axis=1).astype(np.float16)
        )
        bq = bqkv[cols] * SCALE
        bk = bqkv[EMB:][cols]
        b_c = np.concatenate([bq, bk])  # [512]; v bias handled on host
        b_c = np.ascontiguousarray(b_c.reshape(4, 128).T)  # [128, 4]
        wp_c = np.ascontiguousarray(Wproj[cols, :].astype(np.float16))
        xT_c = np.ascontiguousarray(x[b].T.astype(np.float16))  # [1024, 2048]
        in_maps.append({"xT": xT_c, "wqkv": w_c, "bqkv": b_c, "wproj": wp_c})
    return in_maps


def run(inputs, trace=False, **kwargs):
    """Build (cached), run on 8 cores, return (full_output, BassKernelResults)."""
    from concourse.bass_utils import run_bass_kernel_spmd

    if _compiled.get(1) is None:
        _compiled[1] = _build()
    in_maps = _shard_inputs(
        inputs["x"], inputs["Wqkv"], inputs["bqkv"], inputs["Wproj"]
    )
    res = run_bass_kernel_spmd(
        _compiled[1], in_maps, core_ids=list(range(NCORES)), trace=trace, **kwargs
    )
    partials = np.stack([res.results[c]["out"] for c in range(NCORES)])  # [8,T,EMB]
    bqkv_f = np.asarray(inputs["bqkv"], dtype=np.float64)
    wproj_f = np.asarray(inputs["Wproj"], dtype=np.float64)
    bias = (
        np.asarray(inputs["bproj"], dtype=np.float64)
        + bqkv_f[2 * EMB :] @ wproj_f
    ).astype(np.float32)
    full = np.stack(
        [partials[b * GROUPS : (b + 1) * GROUPS].sum(axis=0) for b in range(B)]
    ) + bias
    return full.astype(np.float32), res


def kernel(**inputs):
    out, _ = run(inputs)
    return out


# revision 13
# speedup vs baseline: 1.0445x; 1.0445x over previous
"""Causal self-attention (B=2, T=2048, EMB=1024, 16 heads) on 8 TRN2 NeuronCores.

Sharding: core c handles batch c//4 and heads [4*(c%4), 4*(c%4)+4).
 - Wqkv is split column-wise per head group (q part pre-scaled by 1/sqrt(hd)),
 - Wproj is split row-wise per head group,
 - each core emits a partial [2048, 1024] projection output,
 - host sums the 4 partials per batch and adds bproj + bv@Wproj
   (softmax rows sum to 1, so the v bias contributes a constant row vector
   that the host can add; the device kernel drops bv entirely).

Device kernel (per core, SPMD):
 - host supplies x^T so both qkv matmul operands have the contraction on
   partitions; qkT is produced directly in [qkv_col, token] (transposed)
   layout. v is produced token-major directly (lhsT = x^T token tile), no
   PE transposes needed, and lands next to a ones column (row-sum trick
   for the softmax denominator).
 - attention runs in the S^T = (K Q^T) layout, chunk-major. Within a chunk
   the two head-pair units are interleaved j-tile by j-tile so four
   independent S->exp chains are always in flight; PV matmuls trail one
   block of 4 j-tiles as contiguous per-head accumulation chains (avoids
   the HW per-matmul PSUM-group-switch cost).
 - at chunk end each [65, 512] PV accumulator (row 64 = softmax denominator
   via the ones column) is evacuated to SBUF, freeing all four PSUM banks
   for the next chunk while the normalize/projection epilogue is deferred.
 - deferred work (next chunk's stage-1 pieces first, then epilogue
   normalize + projection pieces) drains one piece per j-step through the
   attention loops, keeping PE busy during exp waits.
 - softmax denominator DMAs (partition spread for the 128-lane reciprocal)
   ride the gpsimd/Pool SWDGE queue so they never block the activation queue.

All matmul operands are f16 (~1e-4 rel err); PSUM accumulation in f32.
"""
import sys

sys.path.insert(0, "/opt/trn_rl_repo")

import numpy as np

B = 2
T = 2048
EMB = 1024
HEADS = 16
HD = EMB // HEADS  # 64
NCORES = 8
GROUPS = 4                 # head groups (cores per batch)
HPC = HEADS // GROUPS      # 4 heads per core
CQ = HPC * HD              # 256 q (or k or v) columns per core
KT = EMB // 128            # 8 contraction tiles
TCH = 512                  # token chunk
NCH = T // TCH             # 4 chunks
NTT = T // 128             # 16 token tiles
NR = CQ // 128             # 2 head-dim row tiles (= head pairs)
SCALE = HD ** -0.5

_compiled = {}
ABLATE = None  # None | 's1' (stage1 only) | 's12' (no projection)


def _build(loop=1):
    import concourse.bass as bass
    import concourse.tile as tile
    from concourse import bacc, mybir

    F32 = mybir.dt.float32
    F16 = mybir.dt.float16
    AF = mybir.ActivationFunctionType

    nc = bacc.Bacc(None, target_bir_lowering=False)
    xT = nc.dram_tensor("xT", [EMB, T], F16, kind="ExternalInput")
    wqkv = nc.dram_tensor("wqkv", [EMB, 3 * CQ], F16, kind="ExternalInput")
    bqkv = nc.dram_tensor("bqkv", [128, 4], F32, kind="ExternalInput")
    wproj = nc.dram_tensor("wproj", [CQ, EMB], F16, kind="ExternalInput")
    out = nc.dram_tensor("out", [T, EMB], F32, kind="ExternalOutput")

    xT_r = xT.rearrange("(kt p) t -> p kt t", p=128)
    wqkv_r = wqkv.rearrange("(kt p) c -> p kt c", p=128)
    wproj_r = wproj.rearrange("(r p) e -> p r e", p=128)

    with tile.TileContext(nc) as tc:
        with (
            tc.tile_pool(name="const", bufs=1) as const,
            tc.tile_pool(name="qk", bufs=1) as qkp,
            tc.tile_pool(name="xt", bufs=3) as xtp,
            tc.tile_pool(name="pt", bufs=20) as ptp,
            tc.tile_pool(name="oh", bufs=1) as ohp,
            tc.tile_pool(name="csb", bufs=8) as csbp,
            tc.tile_pool(name="den", bufs=5) as denp,
            tc.tile_pool(name="osb", bufs=3) as osbp,
            tc.tile_pool(name="ps", bufs=4, space="PSUM") as psS,
            tc.tile_pool(name="psO", bufs=4, space="PSUM") as psO,
        ):
            # ---- constants ----
            # weights on the scalar HWDGE queue, per k-tile, so the sync
            # queue's xt chunk loads run in parallel and matmuls start early
            bias_sb = const.tile([128, 4], F32)
            nc.scalar.dma_start(out=bias_sb, in_=bqkv[:, :])
            w_sb = const.tile([128, KT, 3 * CQ], F16)
            for kt in range(KT):
                nc.scalar.dma_start(
                    out=w_sb[:, kt, 0:CQ], in_=wqkv_r[:, kt, 0:CQ]
                )
            for cp in range(1, 3):
                nc.scalar.dma_start(
                    out=w_sb[:, :, cp * CQ : (cp + 1) * CQ],
                    in_=wqkv_r[:, :, cp * CQ : (cp + 1) * CQ],
                )
            # stage-3 weights loaded inside body() after the xt chunks
            wp_sb = const.tile([128, NR, EMB], F16)
            tri_f = const.tile([128, 128], F32)
            nc.gpsimd.memset(tri_f, 1.0)
            # keep where i(free) >= j(partition): -j + i >= 0
            nc.gpsimd.affine_select(
                out=tri_f, in_=tri_f,
                compare_op=mybir.AluOpType.is_ge,
                fill=0.0, base=0,
                pattern=[[1, 128]], channel_multiplier=-1,
            )
            tri = const.tile([128, 128], F16)
            nc.vector.tensor_copy(tri, tri_f)
            ones_f = const.tile([128, 64], F32)
            nc.vector.memset(ones_f, 1.0)
            ones64 = const.tile([1, 64], F16)
            nc.vector.tensor_copy(ones64, ones_f[0:1, :])
            # v in token-major, per (token_tile, head): 64 cols + ones col
            v_sb = const.tile([128, NTT, HPC, HD + 1], F16)
            nc.vector.tensor_copy(
                out=v_sb[:, :, :, HD : HD + 1],
                in_=ones_f.rearrange("p (a b c) -> p a b c", a=NTT, b=HPC),
            )
            qkT_sb = qkp.tile([128, 4, T], F16)
            ohT = ohp.tile([128, NR, T], F16)

            def body():
                # two-priority deferred-work queues: (chunk, fn) stage-1
                # pieces first, then epilogue pieces
                q_s1 = []
                q_epi = []

                def emit_filler(n=1):
                    for _ in range(n):
                        if q_s1:
                            q_s1.pop(0)[1]()
                        elif q_epi:
                            q_epi.pop(0)()
                        else:
                            return

                def force_s1(upto_ch):
                    while q_s1 and q_s1[0][0] <= upto_ch:
                        q_s1.pop(0)[1]()

                # ---- stage 1: qkv projection ----
                # qkT_sb[:, cb, t]: cb 0,1 = q col-tiles, 2,3 = k col-tiles
                # (transposed layout); v goes token-major straight into v_sb
                def stage1_chunk(ch, inline):
                    xt = xtp.tile([128, KT, TCH], F16)
                    if ch == 0:
                        # per k-tile loads so the first matmul starts after
                        # the first [128, 512] tile instead of the full chunk
                        for kt in range(KT):
                            nc.sync.dma_start(
                                out=xt[:, kt, :],
                                in_=xT_r[:, kt, ch * TCH : (ch + 1) * TCH],
                            )
                    else:
                        nc.sync.dma_start(
                            out=xt, in_=xT_r[:, :, ch * TCH : (ch + 1) * TCH]
                        )

                    def make_qk(cb):
                        def qk_piece():
                            ps = psS.tile([128, TCH], mybir.dt.float32, tag="ps")
                            for kt in range(KT):
                                nc.tensor.matmul(
                                    ps,
                                    w_sb[:, kt, cb * 128 : (cb + 1) * 128],
                                    xt[:, kt, :],
                                    start=(kt == 0),
                                    stop=(kt == KT - 1),
                                )
                            nc.vector.tensor_scalar_add(
                                qkT_sb[:, cb, ch * TCH : (ch + 1) * TCH],
                                ps,
                                bias_sb[:, cb : cb + 1],
                            )
                        return qk_piece

                    def make_v(s):
                        def v_piece():
                            tt = ch * (TCH // 128) + s
                            psv = psS.tile([128, CQ], mybir.dt.float32, tag="ps")
                            for kt in range(KT):
                                nc.tensor.matmul(
                                    psv,
                                    xt[:, kt, s * 128 : (s + 1) * 128],
                                    w_sb[:, kt, 2 * CQ : 3 * CQ],
                                    start=(kt == 0),
                                    stop=(kt == KT - 1),
                                )
                            nc.vector.tensor_copy(
                                v_sb[:, tt, :, 0:HD],
                                psv.rearrange("p (h d) -> p h d", h=HPC),
                            )
                        return v_piece

                    pieces = [make_qk(cb) for cb in range(4)]
                    pieces += [make_v(s) for s in range(TCH // 128)]
                    if inline:
                        for p in pieces:
                            p()
                    else:
                        q_s1.extend((ch, p) for p in pieces)

                def make_partB_rp(rec_rows, holder):
                    def partB_rp():
                        # packed reciprocal broadcast: head-even -> psum rows
                        # 0:64, head-odd -> rows 64:128
                        rp = psS.tile([128, TCH], mybir.dt.float32, tag="ps")
                        nc.tensor.matmul(
                            rp[0:64, :], ones64, rec_rows[0], start=True, stop=True
                        )
                        nc.tensor.matmul(
                            rp[64:128, :], ones64, rec_rows[1], start=True, stop=True
                        )
                        rec_sb = denp.tile([64, 2, TCH], F32, tag="rec_sb")
                        nc.vector.tensor_copy(rec_sb[:, 0, :], rp[0:64, :])
                        nc.vector.tensor_copy(rec_sb[:, 1, :], rp[64:128, :])
                        holder.append(rec_sb)
                    return partB_rp

                def make_partB_mul(cs2, holder, r, base):
                    def partB_mul():
                        rec_sb = holder[0]
                        nc.vector.tensor_mul(
                            ohT[0:64, r, base : base + TCH],
                            cs2[0][0:64, :],
                            rec_sb[:, 0, :],
                        )
                        nc.vector.tensor_mul(
                            ohT[64:128, r, base : base + TCH],
                            cs2[1][0:64, :],
                            rec_sb[:, 1, :],
                        )
                    return partB_mul

                def make_proj(tt, nn):
                    def proj_piece():
                        pp = psS.tile([128, TCH], mybir.dt.float32, tag="ps")
                        for r2 in range(NR):
                            nc.tensor.matmul(
                                pp,
                                ohT[:, r2, tt * 128 : (tt + 1) * 128],
                                wp_sb[:, r2, nn * 512 : (nn + 1) * 512],
                                start=(r2 == 0),
                                stop=(r2 == NR - 1),
                            )
                        osb = osbp.tile([128, 512], F32)
                        nc.any.tensor_copy(osb, pp)
                        nc.sync.dma_start(
                            out=out[
                                tt * 128 : (tt + 1) * 128,
                                nn * 512 : (nn + 1) * 512,
                            ],
                            in_=osb,
                        )
                    return proj_piece

                def emit_chunk(cc):
                    base = cc * TCH
                    jmax = 4 * cc + 3
                    diag = [j for j in range(4 * cc, jmax + 1) if j != 0]
                    rest = [j for j in range(1, 4 * cc)]
                    order = [0] + diag + rest
                    jlast = order[-1]
                    psC = [
                        [
                            psO.tile(
                                [65, TCH], mybir.dt.float32, tag="psO",
                                name=f"psC_{cc}_{_r}_{_hh}",
                            )
                            for _hh in range(2)
                        ]
                        for _r in range(NR)
                    ]
                    blks = [[] for _ in range(NR)]

                    def pv_chain(r, blk):
                        for hh in range(2):
                            for jt, pts, lo, hi in blk:
                                nc.tensor.matmul(
                                    psC[r][hh][:, lo - base : hi - base],
                                    v_sb[:, jt, 2 * r + hh, :],
                                    pts[hh][:, 0 : hi - lo],
                                    start=(jt == 0),
                                    stop=(jt == jlast),
                                    skip_group_check=(jt != 0),
                                )

                    for pos, jt in enumerate(order):
                        i0 = 128 * jt
                        lo = max(base, i0)
                        hi = base + TCH
                        w = hi - lo
                        for r in range(NR):
                            pts = []
                            for hh in range(2):
                                po = 64 * hh
                                sp = psS.tile(
                                    [128, TCH], mybir.dt.float32, tag="ps"
                                )
                                nc.tensor.matmul(
                                    sp[:, 0:w],
                                    qkT_sb[po : po + 64, 2 + r, i0 : i0 + 128],
                                    qkT_sb[po : po + 64, r, lo:hi],
                                    start=True,
                                    stop=True,
                                )
                                pt = ptp.tile([128, TCH], F16)
                                nc.scalar.activation(
                                    pt[:, 0:w], sp[:, 0:w], AF.Exp
                                )
                                if i0 >= base:  # diagonal block: causal mask
                                    nc.vector.tensor_mul(
                                        pt[:, 0:128], pt[:, 0:128], tri
                                    )
                                pts.append(pt)
                            blks[r].append((jt, pts, lo, hi))
                            if len(blks[r]) == 4 and pos < len(order) - 1:
                                pv_chain(r, blks[r])
                                blks[r] = []
                        emit_filler(1)
                    for r in range(NR):
                        pv_chain(r, blks[r])
                    # evacuate accumulators to SBUF (frees PSUM), then the
                    # denominator chains (no PE work; DMAs on the Pool queue,
                    # reading the den row straight from PSUM so they start in
                    # parallel with the evacuation copy)
                    all_cs2 = []
                    all_rec = []
                    for r in range(NR):
                        cs2 = []
                        rec_rows = []
                        for hh in range(2):
                            csb = csbp.tile([65, TCH], F32)
                            nc.vector.tensor_copy(csb, psC[r][hh])
                            cs2.append(csb)
                            den128 = denp.tile([128, TCH // 128], F32, tag="den128")
                            nc.gpsimd.dma_start(out=den128, in_=csb[64:65, :])
                            rec128 = denp.tile([128, TCH // 128], F32, tag="rec128")
                            nc.vector.reciprocal(rec128, den128)
                            rec16 = denp.tile([128, TCH // 128], F16, tag="rec16")
                            nc.vector.tensor_copy(rec16, rec128)
                            rec_row = denp.tile([1, TCH], F16, tag="rec_row")
                            nc.gpsimd.dma_start(out=rec_row, in_=rec16)
                            rec_rows.append(rec_row)
                        all_cs2.append(cs2)
                        all_rec.append(rec_rows)
                    for r in range(NR):
                        holder = []
                        q_epi.append(make_partB_rp(all_rec[r], holder))
                        q_epi.append(make_partB_mul(all_cs2[r], holder, r, base))
                    if ABLATE != "s12":
                        for tt in range(4 * cc, 4 * cc + 4):
                            for nn in range(EMB // 512):
                                q_epi.append(make_proj(tt, nn))

                # ---- emission schedule ----
                stage1_chunk(0, inline=True)
                if ABLATE == "s1":
                    for ch in range(1, NCH):
                        stage1_chunk(ch, inline=True)
                    return
                stage1_chunk(1, inline=False)
                nc.sync.dma_start(out=wp_sb, in_=wproj_r)
                emit_chunk(0)
                stage1_chunk(2, inline=False)
                force_s1(1)
                emit_chunk(1)
                stage1_chunk(3, inline=False)
                force_s1(2)
                emit_chunk(2)
                force_s1(3)
                emit_chunk(3)
                while q_s1 or q_epi:
                    emit_filler(1)

            if loop == 1:
                body()
            else:
                with tc.For_i(
                    0, loop, 1,
                    hint_engines=(
                        mybir.EngineType.PE,
                        mybir.EngineType.Activation,
                        mybir.EngineType.DVE,
                        mybir.EngineType.SP,
                        mybir.EngineType.Pool,
                    ),
                ):
                    body()

    nc.finalize()
    return nc


def _shard_inputs(x, Wqkv, bqkv, Wproj):
    """Build the 8 per-core input maps."""
    x = np.asarray(x, dtype=np.float32)
    Wqkv = np.asarray(Wqkv, dtype=np.float32)
    bqkv = np.asarray(bqkv, dtype=np.float32)
    Wproj = np.asarray(Wproj, dtype=np.float32)

    in_maps = []
    for c in range(NCORES):
        b = c // GROUPS
        g = c % GROUPS
        cols = slice(g * CQ, (g + 1) * CQ)
        wq = Wqkv[:, cols] * SCALE
        wk = Wqkv[:, EMB:][:, cols]
        wv = Wqkv[:, 2 * EMB:][:, cols]
        w_c = np.ascontiguousarray(
            np.concatenate([wq, wk, wv], axis=1).astype(np.float16)
        )
        bq = bqkv[cols] * SCALE
        bk = bqkv[EMB:][cols]
        b_c = np.concatenate([bq, bk])  # [512]; v bias handled on host
        b_c = np.ascontiguousarray(b_c.reshape(4, 128).T)  # [128, 4]
        wp_c = np.ascontiguousarray(Wproj[cols, :].astype(np.float16))
        xT_c = np.ascontiguousarray(x[b].T.astype(np.float16))  # [1024, 2048]
        in_maps.append({"xT": xT_c, "wqkv": w_c, "bqkv": b_c, "wproj": wp_c})
    return in_maps


def run(inputs, trace=False, **kwargs):
    """Build (cached), run on 8 cores, return (full_output, BassKernelResults)."""
    from concourse.bass_utils import run_bass_kernel_spmd

    if _compiled.get(1) is None:
        _compiled[1] = _build()
    in_maps = _shard_inputs(
        inputs["x"], inputs["Wqkv"], inputs["bqkv"], inputs["Wproj"]
    )
    res = run_bass_kernel_spmd(
        _compiled[1], in_maps, core_ids=list(range(NCORES)), trace=trace, **kwargs
    )
    partials = np.stack([res.results[c]["out"] for c in range(NCORES)])  # [8,T,EMB]
    bqkv_f = np.asarray(inputs["bqkv"], dtype=np.float64)
    wproj_f = np.asarray(inputs["Wproj"], dtype=np.float64)
    bias = (
        np.asarray(inputs["bproj"], dtype=np.float64)
        + bqkv_f[2 * EMB :] @ wproj_f
    ).astype(np.float32)
    full = np.stack(
        [partials[b * GROUPS : (b + 1) * GROUPS].sum(axis=0) for b in range(B)]
    ) + bias
    return full.astype(np.float32), res


def kernel(**inputs):
    out, _ = run(inputs)
    return out
